# revision 19
# baseline (speedup 1.0000x reference)
"""Trainium2 Bass kernel for nn_AttentionLayer (B=2, S=2048, D=1024, H=16, dh=64).

Sharding: head-parallel across 8 NeuronCores — each core computes the Q/K/V
projections for its 2 heads (column slices of Wq/Wk/Wv), causal attention for
its 4 (batch, head) pairs, then an AllToAll exchanges per-head context so each
core runs the output projection for 1/8 of the tokens.

All matmuls run in bf16 (full PE rate + fast weight load); PSUM accumulation
stays fp32. Softmax skips the max-subtraction (|scores| <= 8 after the
1/sqrt(64) scale, since q/k are tanh outputs), so exp is a single ACT pass and
row sums come from an appended ones-column in the alpha @ V matmul.

v5 structure:
- q/k live batch-packed in partitions (p = 64*b + d): the two batches' K=64
  score matmuls of a key tile land in disjoint PE row groups and run
  concurrently (~2x score throughput).
- Attention inner loop: supergroups of 3 key tiles (slots of a persistent
  6-bank score PSUM tensor); exp trails the scores by a group so the PE's
  busy stretches stay long (finer-grained skewing measurably re-throttles
  the PE clock via HAM).
- The flush reciprocal-broadcast matmul rides the score-slot rotation instead
  of the ctx-accumulator banks, so deferred flushes actually run early instead
  of stalling on a PSUM WAR against the live ctx accumulation.
- Output projection contracts K=128 by stacking source-core pairs of the A2A
  result in partitions (host-permuted Wo); the h=0 half is issued right after
  the A2A#2 trigger to hide the collective.
- A warm-up AllToAll on the real a2a tensors as the FIRST gpsimd instruction
  absorbs inter-core launch skew AND pre-warms the CC mesh algo: the first
  collective of a shape pays ~11us of setup, repeats ~1us.
- All bulk phase-1 DMA stays on the sync queue (hardware DGE, sprays across
  16 engines); gpsimd's software DGE carries only wo/bo and the post-A2A
  context loads — software DMAs are ~0.6us each and block the gpsimd queue,
  which also carries the collective triggers.
- Each dma_start costs ~0.65us of sequencer dispatch, so bulk transfers are
  single strided-AP DMAs: one per statesT token tile (8 k-slices at once),
  one per weight tensor, one per flush (both batches' a2a blocks), one per
  A2A result head. Dispatch count, not bandwidth, set the old startup time.

Self-contained: accepts the full unsharded inputs, returns the full output.
"""

import numpy as np

import concourse.bass as bass
import concourse.mybir as mybir
import concourse.tile as tile
from concourse import bacc
from concourse.bass_utils import run_bass_kernel_spmd

B, S, D = 2, 2048, 1024
H, DH = 16, 64
N_CORES = 8
HPC = H // N_CORES          # heads per core (2)
LC = HPC * DH               # local projection columns (128)
T = B * S                   # total tokens (4096)
TBLK = T // N_CORES         # tokens per output block (512)

f32 = mybir.dt.float32
bf16 = mybir.dt.bfloat16

_CACHE = {}
LAST_RESULTS = None


def _build():
    nc = bacc.Bacc("TRN2", target_bir_lowering=False, debug=False,
                   num_devices=N_CORES)

    statesT = nc.dram_tensor("statesT", [D, T], bf16, kind="ExternalInput")
    wq = nc.dram_tensor("wq", [D, LC], bf16, kind="ExternalInput")
    wk = nc.dram_tensor("wk", [D, LC], bf16, kind="ExternalInput")
    wv = nc.dram_tensor("wv", [D, LC], bf16, kind="ExternalInput")
    # wo is host-permuted: [h, j, p, o] rows where partition p stacks the
    # (2j, 2j+1) source-core halves of head-pair h -> K=128 out-proj matmuls
    wo = nc.dram_tensor("wo", [D, D], bf16, kind="ExternalInput")
    bq = nc.dram_tensor("bq", [LC, 1], f32, kind="ExternalInput")
    bk = nc.dram_tensor("bk", [LC, 1], f32, kind="ExternalInput")
    bv = nc.dram_tensor("bv", [LC, 1], f32, kind="ExternalInput")
    bo = nc.dram_tensor("bo", [D, 1], f32, kind="ExternalInput")
    tri2 = nc.dram_tensor("tri2", [128, 2 * 128], bf16, kind="ExternalInput")
    ident16 = nc.dram_tensor("ident16", [128, 128], bf16, kind="ExternalInput")
    ones = nc.dram_tensor("ones", [128, 64], bf16, kind="ExternalInput")

    a2a_in = [nc.dram_tensor(f"a2a_in{h}", [N_CORES, DH, TBLK], bf16)
              for h in range(HPC)]
    a2a_out = [nc.dram_tensor(f"a2a_out{h}", [N_CORES, DH, TBLK], bf16)
               for h in range(HPC)]
    bar_out = nc.dram_tensor("bar_out", [N_CORES, DH, TBLK], bf16)
    out = nc.dram_tensor("out", [D, TBLK], f32, kind="ExternalOutput")

    Tanh = mybir.ActivationFunctionType.Tanh
    Exp = mybir.ActivationFunctionType.Exp

    with tile.TileContext(nc) as tc:
        with (
            tc.tile_pool(name="consts", bufs=1) as consts,
            tc.tile_pool(name="persist", bufs=1) as persist,
            tc.tile_pool(name="stream", bufs=4) as stream,
            tc.tile_pool(name="vtp", bufs=2) as vtp,
            tc.tile_pool(name="etp", bufs=7) as etp,
            tc.tile_pool(name="cxp", bufs=2) as cxp,
            tc.tile_pool(name="outp", bufs=3) as outp,
            # PSUM: mm_ps [128,6,512] = 6 banks (3 double-bank score slots,
            # manual rotation), tag "acc" [128,512] x2 = 2 banks -> 8 total
            tc.tile_pool(name="psum", bufs=1, space="PSUM") as psum,
        ):
            mm_ps = psum.tile([128, 6, 512], f32, tag="mm", bufs=1)
            mm_cnt = [0]

            def mm_slot():
                s = mm_cnt[0] % 3
                mm_cnt[0] += 1
                return s

            # ---- constants / weights in SBUF ----
            wq_sb = consts.tile([128, 8, LC], bf16)
            wk_sb = consts.tile([128, 8, LC], bf16)
            wv_sb = consts.tile([128, 8, LC], bf16)
            wq_ap = wq.ap().rearrange("(k p) l -> p k l", p=128)
            wk_ap = wk.ap().rearrange("(k p) l -> p k l", p=128)
            wv_ap = wv.ap().rearrange("(k p) l -> p k l", p=128)
            bq_sb = consts.tile([LC, 1], f32)
            bk_sb = consts.tile([LC, 1], f32)
            bv_sb = consts.tile([LC, 1], f32)
            ident_sb = consts.tile([128, 128], bf16)
            tri2_sb = consts.tile([128, 2, 128], bf16)
            ones_sb = consts.tile([128, 64], bf16)
            # warm-up collective as the very first gpsimd instruction: the CC
            # rendezvous absorbs launch skew and pre-warms the mesh algo for
            # the real A2As while the PE is still DMA-bound
            nc.gpsimd.collective_compute(
                "AllToAll", mybir.AluOpType.bypass,
                replica_groups=[list(range(N_CORES))],
                ins=[a2a_in[0][:].opt()], outs=[bar_out[:].opt()],
            )
            # biases first (768 bytes): the first tanh needs them, and they
            # must not queue behind megabytes of statesT traffic
            nc.sync.dma_start(out=bq_sb, in_=bq[:, :])
            nc.sync.dma_start(out=bk_sb, in_=bk[:, :])
            nc.sync.dma_start(out=bv_sb, in_=bv[:, :])
            # wo/bo are needed only by phase 3; DMA is issued after phase 1's
            # statesT tiles below.
            wo_sb = persist.tile([128, 2, 4, D], bf16)
            bo_sb = consts.tile([128, 8, 1], f32)

            # ---- phase 1: Q/K/V projections ----
            # qt/kt batch-packed: partition p = 64*b + d, free = (h, s).
            # v5: per 128-token tile, [tok_local, (h0 V | ones | h1 V | ones)]
            qt_sb = persist.tile([128, HPC, S], bf16, name="qt")
            kt_sb = persist.tile([128, HPC, S], bf16, name="kt")
            v5_sb = persist.tile([128, T // 128, 2 * (DH + 1)], bf16)

            st_ap = statesT.ap().rearrange("(g p) c -> p g c", g=8)
            for tt in range(T // 1024):  # 4 double-width token tiles
                bb = tt // 2                 # batch of this token tile
                tl = tt % 2                  # position within the batch
                st = stream.tile([128, 8, 1024], bf16, tag="st", bufs=3)
                csl = slice(1024 * tt, 1024 * (tt + 1))
                if tt == 0:
                    # first-matmul critical path: wq, then the first k-slice
                    # of statesT, then the rest — each one strided DMA
                    nc.sync.dma_start(out=wq_sb, in_=wq_ap)
                    nc.sync.dma_start(out=st[:, 0:1, :], in_=st_ap[:, 0:1, csl])
                    nc.sync.dma_start(out=wk_sb, in_=wk_ap)
                    nc.sync.dma_start(out=wv_sb, in_=wv_ap)
                    nc.sync.dma_start(out=st[:, 1:8, :], in_=st_ap[:, 1:8, csl])
                    nc.sync.dma_start(out=ident_sb, in_=ident16[:, :])
                    nc.sync.dma_start(
                        out=tri2_sb,
                        in_=tri2.ap().rearrange("p (a c) -> p a c", a=2))
                    nc.sync.dma_start(out=ones_sb, in_=ones[:, :])
                    nc.vector.tensor_copy(
                        v5_sb.rearrange("p t (a b) -> p (t a) b", a=2)[:, :, DH:DH + 1].opt(),
                        ones_sb[:, :].opt(),
                    )
                else:
                    nc.sync.dma_start(out=st, in_=st_ap[:, :, csl])
                sts = [st[:, kk, :] for kk in range(8)]
                vt_c = vtp.tile([128, 1024], bf16, tag="vt")
                for w_sb, b_sb, kind in (
                        (wq_sb, bq_sb, "q"),
                        (wk_sb, bk_sb, "k"),
                        (wv_sb, bv_sb, "v")):
                    s = mm_slot()
                    acc = mm_ps[:, 2 * s:2 * s + 2, :]
                    for kk in range(8):
                        for half in range(2):
                            nc.tensor.matmul(acc[:, half, :], w_sb[:, kk, :],
                                             sts[kk][:, 512 * half:512 * (half + 1)],
                                             start=(kk == 0), stop=(kk == 7))
                    if kind == "v":
                        nc.scalar.activation(out=vt_c[:, :], in_=acc,
                                             func=Tanh, bias=b_sb)
                    else:
                        dst = qt_sb if kind == "q" else kt_sb
                        tmp = vtp.tile([128, 2, 512], bf16, tag="qk_tmp", bufs=3)
                        nc.scalar.activation(out=tmp, in_=acc,
                                             func=Tanh, bias=b_sb)
                        # repartition (h*64+d -> b*64+d) on the DVE
                        for h in range(HPC):
                            nc.vector.tensor_copy(
                                dst[64 * bb:64 * bb + 64, h,
                                    1024 * tl:1024 * (tl + 1)]
                                .rearrange("p (a c) -> p a c", a=2),
                                tmp[64 * h:64 * h + 64, :, :],
                            )
                # transpose each 128-col block of vt into v5 (both heads at once)
                for j in range(8):
                    t_idx = 8 * tt + j
                    trp = psum.tile([128, 1024], bf16, tag="acc", bufs=2)
                    nc.tensor.transpose(trp[:, 0:128],
                                        vt_c[:, 128 * j:128 * (j + 1)], ident_sb)
                    nc.vector.tensor_copy(
                        v5_sb.rearrange("p t (a b) -> p t a b", a=2)[:, t_idx, :, 0:DH],
                        trp[:, 0:128].rearrange("p (a b) -> p a b", a=2),
                    )

            # wo/bo stream in behind phase 1's statesT tiles, long before
            # phase 3 consumes them
            nc.gpsimd.dma_start(
                out=wo_sb,
                in_=wo.ap().rearrange("(h j p) o -> p h j o", h=2, j=4))
            nc.gpsimd.dma_start(
                out=bo_sb, in_=bo.ap().rearrange("(k p) one -> p k one", p=128))

            # ---- phase 2: causal attention, h-outer for split A2A ----
            # Each key tile's two batch score matmuls are row-packed (b=0 in
            # PE rows 0:63, b=1 in 64:127) and run concurrently. Groups of 3
            # key tiles are software-pipelined: the exp runs a group behind
            # the scores, and two adjacent slots share one ACT op.
            for h in range(HPC):

                def flush_group(grp):
                    # batched 1/l: copy each group's l-row to a distinct
                    # 32-aligned partition, one reciprocal serves them all
                    lb = cxp.tile([64, 512], f32, tag="lb", bufs=1)
                    for i, (cl_sb, _) in enumerate(grp):
                        nc.vector.tensor_copy(lb[32 * i:32 * i + 1, :],
                                              cl_sb[DH:DH + 1, :])
                    rbf = cxp.tile([64, 512], f32, tag="rbf", bufs=1)
                    nc.vector.reciprocal_approx_fast(out=rbf, in_=lb)
                    rbat = cxp.tile([64, 512], bf16, tag="rbat", bufs=1)
                    nc.vector.tensor_copy(rbat, rbf)
                    # the reciprocal broadcast rides the score-slot rotation:
                    # unlike the "acc" banks it is never held across a qi, so
                    # the deferred flush doesn't stall on live ctx accumulators
                    rs = mm_slot()
                    rb2 = mm_ps[:, 2 * rs:2 * rs + 2, :]
                    cx2 = cxp.tile([DH, 2, 512], bf16, tag="cx")
                    for i, (cl_sb, tb_idx) in enumerate(grp):
                        nc.tensor.matmul(rb2[0:DH, i, :],
                                         ones_sb[32 * i:32 * i + 1, :],
                                         rbat[32 * i:32 * i + 1, :],
                                         start=True, stop=True,
                                         tile_position=(32 * i, 0))
                        nc.vector.tensor_mul(cx2[:, i, :], cl_sb[0:DH, :],
                                             rb2[0:DH, i, :])
                    # both batches' a2a blocks (dests qi and qi+4) in one
                    # strided DMA
                    fq = grp[0][1]
                    nc.sync.dma_start(
                        out=a2a_in[h].ap().rearrange("t p c -> p t c")[:, fq::4, :],
                        in_=cx2)

                deferred = None  # previous qi's blocks awaiting normalize
                for qi in range(4):
                    nkt = 4 * qi + 4       # causal kt tiles (128 wide)
                    q_lo = 512 * qi
                    ctxps = [psum.tile([128, 512], f32, tag="acc", bufs=2,
                                       name=f"ctxp_h{h}q{qi}b{b}")
                             for b in range(B)]

                    # supergroups of 3 key tiles: issue 3 tiles' score pairs
                    # back-to-back, then their exps; the ctx pairs trail one
                    # GROUP behind, so the PE alternates long score/ctx
                    # bursts and the ACT exp stream never drains
                    def issue_ctx(ets):
                        for ktj, lo, et in ets:
                            for b in range(B):
                                nc.tensor.matmul(
                                    ctxps[b][0:DH + 1, lo:512],
                                    v5_sb[:, 16 * b + ktj, 65 * h:65 * h + 65],
                                    et[:, b, lo:512],
                                    start=(ktj == 0), stop=(ktj == nkt - 1),
                                )

                    prev_ets = None
                    for g0 in range(0, nkt, 3):
                        tiles = list(range(g0, min(g0 + 3, nkt)))
                        ets = []
                        for ktj in tiles:
                            m = ktj - 4 * qi
                            lo = 128 * m if m > 0 else 0
                            s = mm_slot()
                            for b in range(B):
                                nc.tensor.matmul(
                                    mm_ps[:, 2 * s + b, lo:512],
                                    kt_sb[64 * b:64 * b + 64, h,
                                          128 * ktj:128 * ktj + 128],
                                    qt_sb[64 * b:64 * b + 64, h,
                                          q_lo + lo:q_lo + 512],
                                    start=True, stop=True,
                                )
                            et = etp.tile([128, 2, 512], bf16, tag="et")
                            nc.scalar.activation(out=et[:, :, lo:512],
                                                 in_=mm_ps[:, 2 * s:2 * s + 2, lo:512],
                                                 func=Exp, scale=0.125)
                            if m >= 0:  # multiplicative causal mask, both b
                                dlo = 128 * m
                                nc.vector.tensor_mul(
                                    et[:, :, dlo:dlo + 128],
                                    et[:, :, dlo:dlo + 128], tri2_sb)
                            ets.append((ktj, lo, et))
                        if prev_ets is not None:
                            issue_ctx(prev_ets)
                        if g0 == 0 and deferred is not None:
                            flush_group(deferred)
                            deferred = None
                        prev_ets = ets
                    issue_ctx(prev_ets)

                    group = []
                    for b in range(B):
                        # copy ctx+l out of PSUM eagerly (PSUM slot recycles)
                        cl_sb = cxp.tile([DH + 1, 512], f32, tag="cl", bufs=5)
                        nc.vector.tensor_copy(cl_sb, ctxps[b][0:DH + 1, :])
                        group.append((cl_sb, 4 * b + qi))
                    if qi == 3:            # last qi gates the A2A: no defer
                        flush_group(group)
                    else:
                        deferred = group
                # per-head exchange: h=0 overlaps h=1 compute
                nc.gpsimd.collective_compute(
                    "AllToAll", mybir.AluOpType.bypass,
                    replica_groups=[list(range(N_CORES))],
                    ins=[a2a_in[h][:].opt()], outs=[a2a_out[h][:].opt()],
                )
                if h == 0:
                    # h0 context lands mid-h1-attention; park it in SBUF
                    # kc-pair-stacked for the K=128 out-proj matmuls
                    # (one strided DMA for all 8 source blocks)
                    cxt0 = outp.tile([128, 4, 512], bf16, tag="cxt0", bufs=1)
                    nc.gpsimd.dma_start(
                        out=cxt0,
                        in_=a2a_out[0].ap().rearrange(
                            "(j g) d c -> (g d) j c", g=2))
                    cxt0s = [cxt0[:, j, :] for j in range(4)]

            # ---- phase 3: output projection ----
            # h0 half is issued right after the A2A#2 trigger: its 32 K=128
            # matmuls hide the collective's latency.
            s0s = []
            for oc in range(8):
                op0 = psum.tile([128, 512], f32, tag="acc", bufs=2)
                osl = slice(128 * oc, 128 * (oc + 1))
                for j in range(4):
                    nc.tensor.matmul(op0, wo_sb[:, 0, j, osl], cxt0s[j],
                                     start=(j == 0), stop=(j == 3))
                s0 = outp.tile([128, 512], f32, tag="s0", bufs=8)
                nc.vector.tensor_copy(s0, op0)
                s0s.append(s0)
            cxt1 = outp.tile([128, 4, 512], bf16, tag="cxt1", bufs=1)
            nc.sync.dma_start(
                out=cxt1,
                in_=a2a_out[1].ap().rearrange("(j g) d c -> (g d) j c", g=2))
            cxt1s = [cxt1[:, j, :] for j in range(4)]
            for oc in range(8):  # h=1 half after A2A#2, then combine
                op1 = psum.tile([128, 512], f32, tag="acc", bufs=2)
                osl = slice(128 * oc, 128 * (oc + 1))
                for j in range(4):
                    nc.tensor.matmul(op1, wo_sb[:, 1, j, osl], cxt1s[j],
                                     start=(j == 0), stop=(j == 3))
                s1 = outp.tile([128, 512], f32, tag="s1", bufs=2)
                nc.vector.tensor_add(s1, s0s[oc], op1)
                osb = outp.tile([128, 512], f32, tag="osb", bufs=2)
                nc.scalar.activation(out=osb, in_=s1, func=Tanh, bias=bo_sb[:, oc, :])
                nc.sync.dma_start(out=out[osl, :], in_=osb)

    nc.compile()
    return nc


def _get_nc():
    if "nc" not in _CACHE:
        _CACHE["nc"] = _build()
    return _CACHE["nc"]


def kernel(states, Wq, bq, Wk, bk, Wv, bv, Wo, bo):
    global LAST_RESULTS
    import ml_dtypes
    bf = ml_dtypes.bfloat16
    states = np.asarray(states, dtype=np.float32)
    Wq, Wk, Wv, Wo = (np.asarray(w, dtype=np.float32) for w in (Wq, Wk, Wv, Wo))
    bq, bk, bv, bo = (np.asarray(x, dtype=np.float32) for x in (bq, bk, bv, bo))

    statesT = np.ascontiguousarray(states.reshape(T, D).T).astype(bf)
    # tri2[k, (b, c)] = 0 where query column c (within the diagonal
    # 128-block) is strictly left of key row k, else 1 — multiplicative
    # causal mask, duplicated for the two batch slots
    k_idx = np.arange(128)[:, None]
    c_idx = np.arange(128)[None, :]
    tri01 = np.where(c_idx >= k_idx, 1.0, 0.0).astype(np.float32)
    tri2 = np.concatenate([tri01, tri01], axis=1).astype(bf)
    ident16 = np.eye(128, dtype=np.float32).astype(bf)
    ones = np.ones((128, 64), dtype=np.float32).astype(bf)

    # out-proj row permutation: global feature row f = head*64 + d with
    # head = 2*kc + h; partition p of tile (h, j) stacks kc=2j (p<64)
    # and kc=2j+1 (p>=64)
    wo_r = Wo.reshape(N_CORES, HPC, DH, D)
    wo_packed = np.empty((HPC, 4, 128, D), dtype=np.float32)
    for h in range(HPC):
        for j in range(4):
            wo_packed[h, j, 0:64, :] = wo_r[2 * j, h]
            wo_packed[h, j, 64:128, :] = wo_r[2 * j + 1, h]
    wo_packed = np.ascontiguousarray(wo_packed.reshape(D, D)).astype(bf)

    in_maps = []
    for c in range(N_CORES):
        sl = slice(LC * c, LC * (c + 1))
        in_maps.append({
            "statesT": statesT,
            "wq": np.ascontiguousarray(Wq[:, sl]).astype(bf),
            "wk": np.ascontiguousarray(Wk[:, sl]).astype(bf),
            "wv": np.ascontiguousarray(Wv[:, sl]).astype(bf),
            "wo": wo_packed,
            "bq": np.ascontiguousarray(bq[sl]).reshape(LC, 1),
            "bk": np.ascontiguousarray(bk[sl]).reshape(LC, 1),
            "bv": np.ascontiguousarray(bv[sl]).reshape(LC, 1),
            "bo": bo.reshape(D, 1),
            "tri2": tri2,
            "ident16": ident16,
            "ones": ones,
        })

    nc = _get_nc()
    res = run_bass_kernel_spmd(nc, in_maps, core_ids=list(range(N_CORES)))
    LAST_RESULTS = res

    full = np.empty((T, D), dtype=np.float32)
    for c in range(N_CORES):
        full[TBLK * c:TBLK * (c + 1), :] = res.results[c]["out"].T
    return full.reshape(B, S, D)


# revision 20
# speedup vs baseline: 1.4066x; 1.4066x over previous
"""Trainium2 Bass kernel for nn_AttentionLayer (B=2, S=2048, D=1024, H=16, dh=64).

Sharding: head-parallel across 8 NeuronCores — each core computes the Q/K/V
projections for its 2 heads (column slices of Wq/Wk/Wv), causal attention for
its 4 (batch, head) pairs, then an AllToAll exchanges per-head context so each
core runs the output projection for 1/8 of the tokens.

All matmuls run in bf16 (full PE rate + fast weight load); PSUM accumulation
stays fp32. Softmax skips the max-subtraction (|scores| <= 8 after the
1/sqrt(64) scale, since q/k are tanh outputs), so exp is a single ACT pass and
row sums come from an appended ones-column in the alpha @ V matmul.

v5 structure:
- q/k live batch-packed in partitions (p = 64*b + d): the two batches' K=64
  score matmuls of a key tile land in disjoint PE row groups and run
  concurrently (~2x score throughput).
- Attention inner loop: supergroups of 3 key tiles (3 rotating double-bank
  score PSUM tiles); ctx matmuls trail one group behind the scores/exps so
  the PE alternates long bursts and the ACT exp stream never drains.
- The flush reciprocal-broadcast matmul rides the score-slot rotation instead
  of the ctx-accumulator banks, so deferred flushes actually run early instead
  of stalling on a PSUM WAR against the live ctx accumulation.
- Output projection contracts K=128 by stacking source-core pairs of the A2A
  result in partitions (host-permuted Wo); the h=0 half is issued right after
  the A2A#2 trigger to hide the collective.
- A warm-up AllToAll on the real a2a tensors as the FIRST gpsimd instruction
  absorbs inter-core launch skew AND pre-warms the CC mesh algo: the first
  collective of a shape pays ~11us of setup, repeats ~1us.
- All bulk phase-1 DMA stays on the sync queue (hardware DGE, sprays across
  16 engines); gpsimd's software DGE carries only wo/bo and the post-A2A
  context loads — software DMAs are ~0.6us each and block the gpsimd queue,
  which also carries the collective triggers.
- Each dma_start costs ~0.65us of sequencer dispatch, so bulk transfers are
  single strided-AP DMAs: one per statesT token tile (8 k-slices at once),
  one per weight tensor, one per flush (both batches' a2a blocks), one per
  A2A result head. Dispatch count, not bandwidth, set the old startup time.

Self-contained: accepts the full unsharded inputs, returns the full output.
"""

import numpy as np

import concourse.bass as bass
import concourse.mybir as mybir
import concourse.tile as tile
from concourse import bacc
from concourse.bass_utils import run_bass_kernel_spmd

B, S, D = 2, 2048, 1024
H, DH = 16, 64
N_CORES = 8
HPC = H // N_CORES          # heads per core (2)
LC = HPC * DH               # local projection columns (128)
T = B * S                   # total tokens (4096)
TBLK = T // N_CORES         # tokens per output block (512)

f32 = mybir.dt.float32
bf16 = mybir.dt.bfloat16

_CACHE = {}
LAST_RESULTS = None


def _build():
    nc = bacc.Bacc("TRN2", target_bir_lowering=False, debug=False,
                   num_devices=N_CORES)

    statesT = nc.dram_tensor("statesT", [D, T], bf16, kind="ExternalInput")
    wq = nc.dram_tensor("wq", [D, LC], bf16, kind="ExternalInput")
    wk = nc.dram_tensor("wk", [D, LC], bf16, kind="ExternalInput")
    wv = nc.dram_tensor("wv", [D, LC], bf16, kind="ExternalInput")
    # wo is host-permuted: [h, j, p, o] rows where partition p stacks the
    # (2j, 2j+1) source-core halves of head-pair h -> K=128 out-proj matmuls
    wo = nc.dram_tensor("wo", [D, D], bf16, kind="ExternalInput")
    bq = nc.dram_tensor("bq", [LC, 1], f32, kind="ExternalInput")
    bk = nc.dram_tensor("bk", [LC, 1], f32, kind="ExternalInput")
    bv = nc.dram_tensor("bv", [LC, 1], f32, kind="ExternalInput")
    bo = nc.dram_tensor("bo", [D, 1], f32, kind="ExternalInput")
    tri2 = nc.dram_tensor("tri2", [128, 2 * 128], bf16, kind="ExternalInput")
    ident16 = nc.dram_tensor("ident16", [128, 128], bf16, kind="ExternalInput")
    ones = nc.dram_tensor("ones", [128, 64], bf16, kind="ExternalInput")

    a2a_in = [nc.dram_tensor(f"a2a_in{h}", [N_CORES, DH, TBLK], bf16)
              for h in range(HPC)]
    a2a_out = [nc.dram_tensor(f"a2a_out{h}", [N_CORES, DH, TBLK], bf16)
               for h in range(HPC)]
    bar_out = nc.dram_tensor("bar_out", [N_CORES, DH, TBLK], bf16)
    out = nc.dram_tensor("out", [D, TBLK], f32, kind="ExternalOutput")

    Tanh = mybir.ActivationFunctionType.Tanh
    Exp = mybir.ActivationFunctionType.Exp

    with tile.TileContext(nc) as tc:
        with (
            tc.tile_pool(name="consts", bufs=1) as consts,
            tc.tile_pool(name="persist", bufs=1) as persist,
            tc.tile_pool(name="stream", bufs=4) as stream,
            tc.tile_pool(name="vtp", bufs=2) as vtp,
            tc.tile_pool(name="etp", bufs=7) as etp,
            tc.tile_pool(name="cxp", bufs=2) as cxp,
            tc.tile_pool(name="outp", bufs=3) as outp,
            # PSUM: tag "mm" [128,2,512] x3 = 6 banks (projection acc, score
            # tiles, flush broadcast), tag "acc" [128,512] x2 = 2 banks
            tc.tile_pool(name="psum", bufs=1, space="PSUM") as psum,
        ):
            # ---- constants / weights in SBUF ----
            wq_sb = consts.tile([128, 8, LC], bf16)
            wk_sb = consts.tile([128, 8, LC], bf16)
            wv_sb = consts.tile([128, 8, LC], bf16)
            wq_ap = wq.ap().rearrange("(k p) l -> p k l", p=128)
            wk_ap = wk.ap().rearrange("(k p) l -> p k l", p=128)
            wv_ap = wv.ap().rearrange("(k p) l -> p k l", p=128)
            bq_sb = consts.tile([LC, 1], f32)
            bk_sb = consts.tile([LC, 1], f32)
            bv_sb = consts.tile([LC, 1], f32)
            ident_sb = consts.tile([128, 128], bf16)
            tri2_sb = consts.tile([128, 2, 128], bf16)
            ones_sb = consts.tile([128, 64], bf16)
            # warm-up collective as the very first gpsimd instruction: the CC
            # rendezvous absorbs launch skew and pre-warms the mesh algo for
            # the real A2As while the PE is still DMA-bound
            nc.gpsimd.collective_compute(
                "AllToAll", mybir.AluOpType.bypass,
                replica_groups=[list(range(N_CORES))],
                ins=[a2a_in[0][:].opt()], outs=[bar_out[:].opt()],
            )
            # biases first (768 bytes): the first tanh needs them, and they
            # must not queue behind megabytes of statesT traffic
            nc.sync.dma_start(out=bq_sb, in_=bq[:, :])
            nc.sync.dma_start(out=bk_sb, in_=bk[:, :])
            nc.sync.dma_start(out=bv_sb, in_=bv[:, :])
            # wo/bo are needed only by phase 3; DMA is issued after phase 1's
            # statesT tiles below.
            wo_sb = persist.tile([128, 2, 4, D], bf16)
            bo_sb = consts.tile([128, 8, 1], f32)

            # ---- phase 1: Q/K/V projections ----
            # qt/kt batch-packed: partition p = 64*b + d, free = (h, s).
            # v5: per 128-token tile, [tok_local, (h0 V | ones | h1 V | ones)]
            qt_sb = persist.tile([128, HPC, S], bf16, name="qt")
            kt_sb = persist.tile([128, HPC, S], bf16, name="kt")
            v5_sb = persist.tile([128, T // 128, 2 * (DH + 1)], bf16)

            st_ap = statesT.ap().rearrange("(g p) c -> p g c", g=8)
            for tt in range(T // 1024):  # 4 double-width token tiles
                bb = tt // 2                 # batch of this token tile
                tl = tt % 2                  # position within the batch
                st = stream.tile([128, 8, 1024], bf16, tag="st", bufs=3)
                csl = slice(1024 * tt, 1024 * (tt + 1))
                if tt == 0:
                    # first-matmul critical path: wq, then the first k-slice
                    # of statesT, then the rest — each one strided DMA
                    nc.sync.dma_start(out=wq_sb, in_=wq_ap)
                    nc.sync.dma_start(out=st[:, 0:1, :], in_=st_ap[:, 0:1, csl])
                    nc.sync.dma_start(out=wk_sb, in_=wk_ap)
                    nc.sync.dma_start(out=wv_sb, in_=wv_ap)
                    nc.sync.dma_start(out=st[:, 1:8, :], in_=st_ap[:, 1:8, csl])
                    nc.sync.dma_start(out=ident_sb, in_=ident16[:, :])
                    nc.sync.dma_start(
                        out=tri2_sb,
                        in_=tri2.ap().rearrange("p (a c) -> p a c", a=2))
                    nc.sync.dma_start(out=ones_sb, in_=ones[:, :])
                    nc.vector.tensor_copy(
                        v5_sb.rearrange("p t (a b) -> p (t a) b", a=2)[:, :, DH:DH + 1].opt(),
                        ones_sb[:, :].opt(),
                    )
                else:
                    nc.sync.dma_start(out=st, in_=st_ap[:, :, csl])
                sts = [st[:, kk, :] for kk in range(8)]
                vt_c = vtp.tile([128, 1024], bf16, tag="vt")
                for w_sb, b_sb, kind in (
                        (wq_sb, bq_sb, "q"),
                        (wk_sb, bk_sb, "k"),
                        (wv_sb, bv_sb, "v")):
                    acc = psum.tile([128, 2, 512], f32, tag="mm", bufs=3)
                    for kk in range(8):
                        for half in range(2):
                            nc.tensor.matmul(acc[:, half, :], w_sb[:, kk, :],
                                             sts[kk][:, 512 * half:512 * (half + 1)],
                                             start=(kk == 0), stop=(kk == 7))
                    if kind == "v":
                        nc.scalar.activation(out=vt_c[:, :], in_=acc,
                                             func=Tanh, bias=b_sb)
                    else:
                        dst = qt_sb if kind == "q" else kt_sb
                        tmp = vtp.tile([128, 2, 512], bf16, tag="qk_tmp", bufs=3)
                        nc.scalar.activation(out=tmp, in_=acc,
                                             func=Tanh, bias=b_sb)
                        # repartition (h*64+d -> b*64+d) on the DVE
                        for h in range(HPC):
                            nc.vector.tensor_copy(
                                dst[64 * bb:64 * bb + 64, h,
                                    1024 * tl:1024 * (tl + 1)]
                                .rearrange("p (a c) -> p a c", a=2),
                                tmp[64 * h:64 * h + 64, :, :],
                            )
                # transpose each 128-col block of vt into v5 (both heads at once)
                for j in range(8):
                    t_idx = 8 * tt + j
                    trp = psum.tile([128, 1024], bf16, tag="acc", bufs=2)
                    nc.tensor.transpose(trp[:, 0:128],
                                        vt_c[:, 128 * j:128 * (j + 1)], ident_sb)
                    nc.vector.tensor_copy(
                        v5_sb.rearrange("p t (a b) -> p t a b", a=2)[:, t_idx, :, 0:DH],
                        trp[:, 0:128].rearrange("p (a b) -> p a b", a=2),
                    )

            # wo/bo stream in behind phase 1's statesT tiles, long before
            # phase 3 consumes them
            nc.gpsimd.dma_start(
                out=wo_sb,
                in_=wo.ap().rearrange("(h j p) o -> p h j o", h=2, j=4))
            nc.gpsimd.dma_start(
                out=bo_sb, in_=bo.ap().rearrange("(k p) one -> p k one", p=128))

            # ---- phase 2: causal attention, h-outer for split A2A ----
            # Each key tile's two batch score matmuls are row-packed (b=0 in
            # PE rows 0:63, b=1 in 64:127) and run concurrently. Groups of 3
            # key tiles are software-pipelined: the exp runs a group behind
            # the scores, and two adjacent slots share one ACT op.
            for h in range(HPC):

                def flush_group(grp):
                    # batched 1/l: copy each group's l-row to a distinct
                    # 32-aligned partition, one reciprocal serves them all
                    lb = cxp.tile([64, 512], f32, tag="lb", bufs=1)
                    for i, (cl_sb, _) in enumerate(grp):
                        nc.vector.tensor_copy(lb[32 * i:32 * i + 1, :],
                                              cl_sb[DH:DH + 1, :])
                    rbf = cxp.tile([64, 512], f32, tag="rbf", bufs=1)
                    nc.vector.reciprocal_approx_fast(out=rbf, in_=lb)
                    rbat = cxp.tile([64, 512], bf16, tag="rbat", bufs=1)
                    nc.vector.tensor_copy(rbat, rbf)
                    # the reciprocal broadcast rides the score-slot rotation:
                    # unlike the "acc" banks it is never held across a qi, so
                    # the deferred flush doesn't stall on live ctx accumulators
                    rb2 = psum.tile([128, 2, 512], f32, tag="mm", bufs=3)
                    cx2 = cxp.tile([DH, 2, 512], bf16, tag="cx")
                    for i, (cl_sb, tb_idx) in enumerate(grp):
                        nc.tensor.matmul(rb2[0:DH, i, :],
                                         ones_sb[32 * i:32 * i + 1, :],
                                         rbat[32 * i:32 * i + 1, :],
                                         start=True, stop=True,
                                         tile_position=(32 * i, 0))
                        nc.vector.tensor_mul(cx2[:, i, :], cl_sb[0:DH, :],
                                             rb2[0:DH, i, :])
                    # both batches' a2a blocks (dests qi and qi+4) in one
                    # strided DMA
                    fq = grp[0][1]
                    nc.sync.dma_start(
                        out=a2a_in[h].ap().rearrange("t p c -> p t c")[:, fq::4, :],
                        in_=cx2)

                deferred = None  # previous qi's blocks awaiting normalize
                for qi in range(4):
                    nkt = 4 * qi + 4       # causal kt tiles (128 wide)
                    q_lo = 512 * qi
                    ctxps = [psum.tile([128, 512], f32, tag="acc", bufs=2,
                                       name=f"ctxp_h{h}q{qi}b{b}")
                             for b in range(B)]

                    # supergroups of 3 key tiles: issue 3 tiles' score pairs
                    # back-to-back, then their exps; the ctx pairs trail one
                    # GROUP behind, so the PE alternates long score/ctx
                    # bursts and the ACT exp stream never drains
                    def issue_ctx(ets):
                        for ktj, lo, et in ets:
                            for b in range(B):
                                nc.tensor.matmul(
                                    ctxps[b][0:DH + 1, lo:512],
                                    v5_sb[:, 16 * b + ktj, 65 * h:65 * h + 65],
                                    et[:, b, lo:512],
                                    start=(ktj == 0), stop=(ktj == nkt - 1),
                                )

                    prev_ets = None
                    for g0 in range(0, nkt, 3):
                        tiles = list(range(g0, min(g0 + 3, nkt)))
                        ets = []
                        for ktj in tiles:
                            m = ktj - 4 * qi
                            lo = 128 * m if m > 0 else 0
                            stp = psum.tile([128, 2, 512], f32, tag="mm",
                                            bufs=3)
                            for b in range(B):
                                nc.tensor.matmul(
                                    stp[:, b, lo:512],
                                    kt_sb[64 * b:64 * b + 64, h,
                                          128 * ktj:128 * ktj + 128],
                                    qt_sb[64 * b:64 * b + 64, h,
                                          q_lo + lo:q_lo + 512],
                                    start=True, stop=True,
                                )
                            et = etp.tile([128, 2, 512], bf16, tag="et")
                            nc.scalar.activation(out=et[:, :, lo:512],
                                                 in_=stp[:, :, lo:512],
                                                 func=Exp, scale=0.125)
                            if m >= 0:  # multiplicative causal mask, both b
                                dlo = 128 * m
                                nc.vector.tensor_mul(
                                    et[:, :, dlo:dlo + 128],
                                    et[:, :, dlo:dlo + 128], tri2_sb)
                            ets.append((ktj, lo, et))
                        if prev_ets is not None:
                            issue_ctx(prev_ets)
                        if g0 == 0 and deferred is not None:
                            flush_group(deferred)
                            deferred = None
                        prev_ets = ets
                    issue_ctx(prev_ets)

                    group = []
                    for b in range(B):
                        # copy ctx+l out of PSUM eagerly (PSUM slot recycles)
                        cl_sb = cxp.tile([DH + 1, 512], f32, tag="cl", bufs=5)
                        nc.vector.tensor_copy(cl_sb, ctxps[b][0:DH + 1, :])
                        group.append((cl_sb, 4 * b + qi))
                    if qi == 3:            # last qi gates the A2A: no defer
                        flush_group(group)
                    else:
                        deferred = group
                # per-head exchange: h=0 overlaps h=1 compute
                nc.gpsimd.collective_compute(
                    "AllToAll", mybir.AluOpType.bypass,
                    replica_groups=[list(range(N_CORES))],
                    ins=[a2a_in[h][:].opt()], outs=[a2a_out[h][:].opt()],
                )
                if h == 0:
                    # h0 context lands mid-h1-attention; park it in SBUF
                    # kc-pair-stacked for the K=128 out-proj matmuls
                    # (one strided DMA for all 8 source blocks)
                    cxt0 = outp.tile([128, 4, 512], bf16, tag="cxt0", bufs=1)
                    nc.gpsimd.dma_start(
                        out=cxt0,
                        in_=a2a_out[0].ap().rearrange(
                            "(j g) d c -> (g d) j c", g=2))
                    cxt0s = [cxt0[:, j, :] for j in range(4)]

            # ---- phase 3: output projection ----
            # h0 half is issued right after the A2A#2 trigger: its 32 K=128
            # matmuls hide the collective's latency.
            s0s = []
            for oc in range(8):
                op0 = psum.tile([128, 512], f32, tag="acc", bufs=2)
                osl = slice(128 * oc, 128 * (oc + 1))
                for j in range(4):
                    nc.tensor.matmul(op0, wo_sb[:, 0, j, osl], cxt0s[j],
                                     start=(j == 0), stop=(j == 3))
                s0 = outp.tile([128, 512], f32, tag="s0", bufs=8)
                nc.vector.tensor_copy(s0, op0)
                s0s.append(s0)
            cxt1 = outp.tile([128, 4, 512], bf16, tag="cxt1", bufs=1)
            nc.sync.dma_start(
                out=cxt1,
                in_=a2a_out[1].ap().rearrange("(j g) d c -> (g d) j c", g=2))
            cxt1s = [cxt1[:, j, :] for j in range(4)]
            for oc in range(8):  # h=1 half after A2A#2, then combine
                op1 = psum.tile([128, 512], f32, tag="acc", bufs=2)
                osl = slice(128 * oc, 128 * (oc + 1))
                for j in range(4):
                    nc.tensor.matmul(op1, wo_sb[:, 1, j, osl], cxt1s[j],
                                     start=(j == 0), stop=(j == 3))
                s1 = outp.tile([128, 512], f32, tag="s1", bufs=2)
                nc.vector.tensor_add(s1, s0s[oc], op1)
                osb = outp.tile([128, 512], f32, tag="osb", bufs=2)
                nc.scalar.activation(out=osb, in_=s1, func=Tanh, bias=bo_sb[:, oc, :])
                nc.sync.dma_start(out=out[osl, :], in_=osb)

    nc.compile()
    return nc


def _get_nc():
    if "nc" not in _CACHE:
        _CACHE["nc"] = _build()
    return _CACHE["nc"]


def kernel(states, Wq, bq, Wk, bk, Wv, bv, Wo, bo):
    global LAST_RESULTS
    import ml_dtypes
    bf = ml_dtypes.bfloat16
    states = np.asarray(states, dtype=np.float32)
    Wq, Wk, Wv, Wo = (np.asarray(w, dtype=np.float32) for w in (Wq, Wk, Wv, Wo))
    bq, bk, bv, bo = (np.asarray(x, dtype=np.float32) for x in (bq, bk, bv, bo))

    statesT = np.ascontiguousarray(states.reshape(T, D).T).astype(bf)
    # tri2[k, (b, c)] = 0 where query column c (within the diagonal
    # 128-block) is strictly left of key row k, else 1 — multiplicative
    # causal mask, duplicated for the two batch slots
    k_idx = np.arange(128)[:, None]
    c_idx = np.arange(128)[None, :]
    tri01 = np.where(c_idx >= k_idx, 1.0, 0.0).astype(np.float32)
    tri2 = np.concatenate([tri01, tri01], axis=1).astype(bf)
    ident16 = np.eye(128, dtype=np.float32).astype(bf)
    ones = np.ones((128, 64), dtype=np.float32).astype(bf)

    # out-proj row permutation: global feature row f = head*64 + d with
    # head = 2*kc + h; partition p of tile (h, j) stacks kc=2j (p<64)
    # and kc=2j+1 (p>=64)
    wo_r = Wo.reshape(N_CORES, HPC, DH, D)
    wo_packed = np.empty((HPC, 4, 128, D), dtype=np.float32)
    for h in range(HPC):
        for j in range(4):
            wo_packed[h, j, 0:64, :] = wo_r[2 * j, h]
            wo_packed[h, j, 64:128, :] = wo_r[2 * j + 1, h]
    wo_packed = np.ascontiguousarray(wo_packed.reshape(D, D)).astype(bf)

    in_maps = []
    for c in range(N_CORES):
        sl = slice(LC * c, LC * (c + 1))
        in_maps.append({
            "statesT": statesT,
            "wq": np.ascontiguousarray(Wq[:, sl]).astype(bf),
            "wk": np.ascontiguousarray(Wk[:, sl]).astype(bf),
            "wv": np.ascontiguousarray(Wv[:, sl]).astype(bf),
            "wo": wo_packed,
            "bq": np.ascontiguousarray(bq[sl]).reshape(LC, 1),
            "bk": np.ascontiguousarray(bk[sl]).reshape(LC, 1),
            "bv": np.ascontiguousarray(bv[sl]).reshape(LC, 1),
            "bo": bo.reshape(D, 1),
            "tri2": tri2,
            "ident16": ident16,
            "ones": ones,
        })

    nc = _get_nc()
    res = run_bass_kernel_spmd(nc, in_maps, core_ids=list(range(N_CORES)))
    LAST_RESULTS = res

    full = np.empty((T, D), dtype=np.float32)
    for c in range(N_CORES):
        full[TBLK * c:TBLK * (c + 1), :] = res.results[c]["out"].T
    return full.reshape(B, S, D)
